# revision 1
# baseline (speedup 1.0000x reference)
"""MoE (top-2 of 8 experts, SwiGLU MLP) on 8 Trainium2 NeuronCores.

Strategy (expert-parallel, host-side routing):
  - Host computes the gate (scores -> top-2 -> softmax) in f64; the rank-2/3
    score gap is >1e-4 for these inputs so selection is rounding-robust.
  - Core e receives the tokens routed to expert e (transposed to [H, C],
    zero-padded to capacity C) plus expert e's w1/w3/w2.
  - Each core runs a SwiGLU MLP:  yT = w2.T @ (silu(w1.T @ xT) * (w3.T @ xT))
    entirely with float32r matmuls (full PE rate at moving-dim >= 256),
    keeping x, act and y resident in SBUF; weights are streamed from HBM
    exactly once.
  - Host scatter-adds the weighted per-expert outputs back to [B, S, H].

Hardcoded problem shapes: x [2, 2048, 1024], E=8 experts, top-2,
w1/w3 [8, 1024, 4096], w2 [8, 4096, 1024].
"""

import math

import numpy as np

import concourse.bass as bass  # noqa: F401  (registers AP machinery)
import concourse.tile as tile
from concourse import bacc, mybir
from concourse.bass_utils import run_bass_kernel_spmd

P = 128
H = 1024
F = 4096
E = 8
TOPK = 2
N_CORES = 8

KO = H // P  # 8 contraction tiles for the up/gate projections
FO = F // P  # 32 intermediate tiles
HO = H // P  # 8 output tiles

F32 = mybir.dt.float32
F32R = mybir.dt.float32r

_NC_CACHE: dict = {}


def _chunks(C: int):
    """Split C evenly into chunk widths in [256, 512] (fp32r full PE rate
    needs a moving dim >= 256; one PSUM bank holds <= 512 fp32)."""
    assert C % 16 == 0
    if C <= 512:
        return [(0, C)]
    n = math.ceil(C / 512)
    base = (C // n) // 8 * 8
    extra = (C - base * n) // 8
    widths = [base + (8 if i < extra else 0) for i in range(n)]
    assert sum(widths) == C and all(256 <= cw <= 512 for cw in widths), (C, widths)
    out, off = [], 0
    for cw in widths:
        out.append((off, cw))
        off += cw
    return out


def _pick_fgroup(C: int) -> int:
    """Largest f-group size whose SBUF footprint fits comfortably."""
    for fg in (16, 8, 4):
        # per-partition bytes: x + y resident (KO+HO)*C*4, act fg*C*4,
        # w13 pool 24KB, w2 pool 2*fg*0.5KB, temps ~16KB
        est = 4 * C * (KO + HO + fg) + 24 * 1024 + fg * 1024 + 16 * 1024
        if est <= 176 * 1024:
            return fg
    return 4


def _build_nc(C: int):
    chunks = _chunks(C)
    FG = _pick_fgroup(C)
    n_groups = FO // FG

    nc = bacc.Bacc("TRN2", target_bir_lowering=False, debug=False,
                   num_devices=N_CORES)
    xT = nc.dram_tensor("xT", [H, C], F32R, kind="ExternalInput").ap()
    w1 = nc.dram_tensor("w1", [H, F], F32R, kind="ExternalInput").ap()
    w3 = nc.dram_tensor("w3", [H, F], F32R, kind="ExternalInput").ap()
    w2 = nc.dram_tensor("w2", [F, H], F32R, kind="ExternalInput").ap()
    yT = nc.dram_tensor("yT", [H, C], F32, kind="ExternalOutput").ap()

    w1_t = w1.rearrange("(ko p) f -> p ko f", p=P)  # [128, KO, F]
    w3_t = w3.rearrange("(ko p) f -> p ko f", p=P)
    w2_t = w2.rearrange("(fo p) m -> p fo m", p=P)  # [128, FO, H]
    xT_t = xT.rearrange("(ko p) c -> p ko c", p=P)  # [128, KO, C]
    yT_t = yT.rearrange("(ho p) c -> p ho c", p=P)  # [128, HO, C]

    with tile.TileContext(nc) as tc:
        with (
            tc.tile_pool(name="xres", bufs=1) as xpool,
            tc.tile_pool(name="yres", bufs=1) as ypool,
            tc.tile_pool(name="actres", bufs=1) as actpool,
            tc.tile_pool(name="w13", bufs=3) as w13pool,
            tc.tile_pool(name="w2p", bufs=2) as w2pool,
            tc.tile_pool(name="tmp", bufs=3) as tmppool,
            tc.tile_pool(name="psh", bufs=3, space="PSUM") as ps_h,
            tc.tile_pool(name="psu", bufs=3, space="PSUM") as ps_u,
            tc.tile_pool(name="psy", bufs=2, space="PSUM") as ps_y,
        ):
            w13_tiles = {}

            def load_w13(fo):
                w1_f = w13pool.tile([P, KO, P], F32R, tag="w1",
                                    name=f"w1_f{fo}")
                nc.sync.dma_start(w1_f[:], w1_t[:, :, fo * P:(fo + 1) * P])
                w3_f = w13pool.tile([P, KO, P], F32R, tag="w3",
                                    name=f"w3_f{fo}")
                nc.sync.dma_start(w3_f[:], w3_t[:, :, fo * P:(fo + 1) * P])
                w13_tiles[fo] = (w1_f, w3_f)

            # first f-tile's weights ahead of the x stream so the PE can
            # start as soon as x[k=0, chunk=0] lands
            load_w13(0)

            # x as independent per-(k, chunk) tiles: matmuls can start as
            # soon as their own slice lands instead of waiting for all of x
            x_sb = [
                [xpool.tile([P, cw], F32R, tag=f"x{k}_{ci}",
                            name=f"x_sb_{k}_{ci}")
                 for ci, (off, cw) in enumerate(chunks)]
                for k in range(KO)
            ]
            for k in range(KO):
                for ci, (off, cw) in enumerate(chunks):
                    nc.sync.dma_start(x_sb[k][ci][:], xT_t[:, k, off:off + cw])
            y_sb = ypool.tile([P, HO, C], F32)
            act_sb = actpool.tile([P, FG, C], F32R)

            for g in range(n_groups):
                f0 = g * FG
                # ---- up + gate projections and SwiGLU for this f-group ----
                for fi in range(FG):
                    fo = f0 + fi
                    if fo not in w13_tiles:
                        load_w13(fo)
                    w1_f, w3_f = w13_tiles.pop(fo)
                    for ci, (off, cw) in enumerate(chunks):
                        h_ps = ps_h.tile([P, 512], F32)
                        u_ps = ps_u.tile([P, 512], F32)
                        for k in range(KO):
                            nc.tensor.matmul(
                                h_ps[:, :cw],
                                w1_f[:, k],
                                x_sb[k][ci][:],
                                start=(k == 0), stop=(k == KO - 1),
                            )
                        for k in range(KO):
                            nc.tensor.matmul(
                                u_ps[:, :cw],
                                w3_f[:, k],
                                x_sb[k][ci][:],
                                start=(k == 0), stop=(k == KO - 1),
                            )
                        s_sb = tmppool.tile([P, 512], F32, tag="silu")
                        nc.scalar.activation(
                            s_sb[:, :cw], h_ps[:, :cw],
                            mybir.ActivationFunctionType.Silu,
                        )
                        nc.vector.tensor_mul(
                            act_sb[:, fi, off:off + cw],
                            s_sb[:, :cw], u_ps[:, :cw],
                        )
                # ---- down projection: y += act_g @ w2[f-group] ----
                for ho in range(HO):
                    w2_h = w2pool.tile([P, FG, P], F32R, tag="w2")
                    nc.sync.dma_start(
                        w2_h[:], w2_t[:, f0:f0 + FG, ho * P:(ho + 1) * P])
                    for off, cw in chunks:
                        y_ps = ps_y.tile([P, 512], F32)
                        for fi in range(FG):
                            nc.tensor.matmul(
                                y_ps[:, :cw],
                                w2_h[:, fi],
                                act_sb[:, fi, off:off + cw],
                                start=(fi == 0), stop=(fi == FG - 1),
                            )
                        if g == 0:
                            nc.vector.tensor_copy(
                                y_sb[:, ho, off:off + cw], y_ps[:, :cw])
                        else:
                            nc.vector.tensor_add(
                                y_sb[:, ho, off:off + cw],
                                y_sb[:, ho, off:off + cw], y_ps[:, :cw])
                        if g == n_groups - 1:
                            # final contribution: store while the remaining
                            # tiles are still accumulating
                            nc.sync.dma_start(yT_t[:, ho, off:off + cw],
                                              y_sb[:, ho, off:off + cw])

    nc.compile()
    return nc


def _route(x, gate_w):
    """Host-side gate: returns token index list and combine weight per expert."""
    xt = x.reshape(-1, H)
    scores = xt.astype(np.float64) @ gate_w.astype(np.float64).T
    ei = np.argsort(-scores, axis=1, kind="stable")[:, :TOPK]  # [T, 2]
    ev = np.take_along_axis(scores, ei, axis=1)                # [T, 2]
    ev = ev - ev.max(axis=1, keepdims=True)
    ew = np.exp(ev)
    ew = ew / ew.sum(axis=1, keepdims=True)                    # softmax [T, 2]
    routes = []
    for e in range(E):
        mask = ei == e                                         # [T, 2]
        toks = np.nonzero(mask.any(axis=1))[0]
        wts = (ew * mask).sum(axis=1)[toks]
        routes.append((toks, wts.astype(np.float32)))
    return routes


def _run(inputs, trace=False, trace_kwargs=None):
    x = np.ascontiguousarray(np.asarray(inputs["x"], dtype=np.float32))
    gate_w = np.asarray(inputs["gate_w"], dtype=np.float32)
    w1 = np.asarray(inputs["w1"], dtype=np.float32)
    w3 = np.asarray(inputs["w3"], dtype=np.float32)
    w2 = np.asarray(inputs["w2"], dtype=np.float32)
    B, S, Hd = x.shape
    assert Hd == H and w1.shape == (E, H, F) and w2.shape == (E, F, H)

    routes = _route(x, gate_w)
    max_count = max(len(toks) for toks, _ in routes)
    C = max(256, math.ceil(max_count / 16) * 16)

    if C not in _NC_CACHE:
        _NC_CACHE[C] = _build_nc(C)
    nc = _NC_CACHE[C]

    xt = x.reshape(-1, H)
    in_maps = []
    for e in range(E):
        toks, _ = routes[e]
        xT_e = np.zeros((H, C), dtype=np.float32)
        xT_e[:, :len(toks)] = xt[toks].T
        in_maps.append({
            "xT": xT_e,
            "w1": np.ascontiguousarray(w1[e]),
            "w3": np.ascontiguousarray(w3[e]),
            "w2": np.ascontiguousarray(w2[e]),
        })

    res = run_bass_kernel_spmd(
        nc, in_maps, core_ids=list(range(N_CORES)),
        trace=trace, trace_kwargs=trace_kwargs or {},
    )

    y = np.zeros((B * S, H), dtype=np.float32)
    for e in range(E):
        toks, wts = routes[e]
        yT_e = res.results[e]["yT"]  # [H, C]
        y[toks] += wts[:, None] * yT_e[:, :len(toks)].T
    return y.reshape(B, S, H), res


def kernel(**inputs):
    y, _ = _run(inputs)
    return y



# revision 2
# speedup vs baseline: 1.2859x; 1.2859x over previous
"""MoE (top-2 of 8 experts, SwiGLU MLP) on 8 Trainium2 NeuronCores.

Strategy (expert-parallel, host-side routing, fp8 DoubleRow matmuls):
  - Host computes the gate (scores -> top-2 -> softmax) in f64; the rank-2/3
    score gap is >1e-4 for these inputs so selection is rounding-robust.
  - Core e receives the tokens routed to expert e (transposed to [H, C],
    zero-padded to capacity C) plus expert e's w1/w3/w2.
  - All matmuls run as fp8e4m3 DoubleRow (two 128-deep k-tiles per
    instruction at 0.5 PE cycles per output row = 4x the fp32r MAC rate).
    Plain fp8 quantization is far too coarse (~6e-2 rel err), so every
    operand is split into hi + lo fp8 parts (lo = fp8 of the quantization
    residual) and each product uses 3 of the 4 cross terms:
        a @ w  ~=  a_hi @ w_hi + a_hi @ w_lo + a_lo @ w_hi
    which lands at ~2e-3 rel err (measured) at 0.75 cycles per 128-deep
    output row -- a ~1.33x PE speedup over the fp32r kernel.
  - Weights are pre-scaled by powers of 2 (w1*64, w3*16, w2*64) so their
    0.02-sigma values sit in e4m3's normal range instead of the subnormal
    range (which is what ruins plain fp8 here).  The w1 scale is removed
    by the Silu activation's input scale; the w3 and w2 scales ride
    through the (linear) down projection and are divided out on the host.
  - Each core streams expert weights from HBM once (25MB fp8 vs 50MB f32),
    keeps x, act, y resident in SBUF; act is quantized on-chip to fp8
    hi + lo at scale 16 (absmax(16*act) ~ 114 < 240 = e4m3 max).
  - Host scatter-adds the weighted per-expert outputs back to [B, S, H].

Hardcoded problem shapes: x [2, 2048, 1024], E=8 experts, top-2,
w1/w3 [8, 1024, 4096], w2 [8, 4096, 1024].
"""

import math

import ml_dtypes
import numpy as np

import concourse.bass as bass  # noqa: F401  (registers AP machinery)
import concourse.tile as tile
from concourse import bacc, mybir
from concourse.bass_utils import run_bass_kernel_spmd

P = 128
H = 1024
F = 4096
E = 8
TOPK = 2
N_CORES = 8

KO = H // P  # 8 contraction tiles for the up/gate projections
FO = F // P  # 32 intermediate tiles
HO = H // P  # 8 output tiles
FG = 16      # f-tiles per down-projection group (FO/FG = 2 groups)

SW1 = 64.0   # w1 pre-scale (removed by the Silu input scale)
SW3 = 16.0   # w3 pre-scale => act is produced at scale 16
SW2 = 64.0   # w2 pre-scale
Y_DESCALE = 1.0 / (SW3 * SW2)  # folded into the host combine weights

F32 = mybir.dt.float32
F8 = mybir.dt.float8e4
FP8 = ml_dtypes.float8_e4m3
DR = mybir.MatmulPerfMode.DoubleRow

_NC_CACHE: dict = {}


def _chunks(C: int):
    """Split C evenly into chunk widths <= 512 (one PSUM bank)."""
    assert C % 16 == 0
    if C <= 512:
        return [(0, C)]
    n = math.ceil(C / 512)
    base = (C // n) // 8 * 8
    extra = (C - base * n) // 8
    widths = [base + (8 if i < extra else 0) for i in range(n)]
    assert sum(widths) == C and all(cw <= 512 for cw in widths), (C, widths)
    out, off = [], 0
    for cw in widths:
        out.append((off, cw))
        off += cw
    return out


def _build_nc(C: int):
    chunks = _chunks(C)
    n_groups = FO // FG
    KP = KO // 2   # DoubleRow k-tile pairs in the up projections
    FP = FG // 2   # DoubleRow f-tile pairs per down-projection group

    nc = bacc.Bacc("TRN2", target_bir_lowering=False, debug=False,
                   num_devices=N_CORES)
    xh = nc.dram_tensor("xh", [H, C], F8, kind="ExternalInput").ap()
    xl = nc.dram_tensor("xl", [H, C], F8, kind="ExternalInput").ap()
    w1h = nc.dram_tensor("w1h", [H, F], F8, kind="ExternalInput").ap()
    w1l = nc.dram_tensor("w1l", [H, F], F8, kind="ExternalInput").ap()
    w3h = nc.dram_tensor("w3h", [H, F], F8, kind="ExternalInput").ap()
    w3l = nc.dram_tensor("w3l", [H, F], F8, kind="ExternalInput").ap()
    w2h = nc.dram_tensor("w2h", [F, H], F8, kind="ExternalInput").ap()
    w2l = nc.dram_tensor("w2l", [F, H], F8, kind="ExternalInput").ap()
    yT = nc.dram_tensor("yT", [H, C], F32, kind="ExternalOutput").ap()

    w13_t = {
        "w1h": w1h.rearrange("(ko p) f -> p ko f", p=P),  # [128, KO, F]
        "w1l": w1l.rearrange("(ko p) f -> p ko f", p=P),
        "w3h": w3h.rearrange("(ko p) f -> p ko f", p=P),
        "w3l": w3l.rearrange("(ko p) f -> p ko f", p=P),
    }
    w2h_t = w2h.rearrange("(fo p) m -> p fo m", p=P)      # [128, FO, H]
    w2l_t = w2l.rearrange("(fo p) m -> p fo m", p=P)
    xh_t = xh.rearrange("(ko p) c -> p ko c", p=P)        # [128, KO, C]
    xl_t = xl.rearrange("(ko p) c -> p ko c", p=P)
    yT_t = yT.rearrange("(ho p) c -> p ho c", p=P)        # [128, HO, C]

    with tile.TileContext(nc) as tc:
        with (
            tc.tile_pool(name="xres", bufs=1) as xpool,
            tc.tile_pool(name="yres", bufs=1) as ypool,
            tc.tile_pool(name="actres", bufs=1) as actpool,
            tc.tile_pool(name="w13", bufs=3) as w13pool,
            tc.tile_pool(name="w2p", bufs=2) as w2pool,
            tc.tile_pool(name="tmp", bufs=3) as tmppool,
            tc.tile_pool(name="psh", bufs=3, space="PSUM") as ps_h,
            tc.tile_pool(name="psu", bufs=3, space="PSUM") as ps_u,
            tc.tile_pool(name="psy", bufs=2, space="PSUM") as ps_y,
        ):
            w13_tiles = {}

            def load_w13(fo):
                tiles = []
                for nm in ("w1h", "w1l", "w3h", "w3l"):
                    t = w13pool.tile([P, KO, P], F8, tag=nm,
                                     name=f"{nm}_f{fo}")
                    nc.sync.dma_start(t[:], w13_t[nm][:, :, fo * P:(fo + 1) * P])
                    tiles.append(t)
                w13_tiles[fo] = tiles

            # two f-tiles of weights ahead of the x stream so the PE can
            # start as soon as x[k-pair 0, chunk 0] lands
            load_w13(0)
            load_w13(1)

            # x as independent per-(k-pair, chunk) tiles: matmuls start as
            # soon as their own slice lands instead of waiting for all of x
            def load_x(src_t, nm):
                sb = [
                    [xpool.tile([P, 2, cw], F8, tag=f"{nm}{kp}_{ci}",
                                name=f"{nm}_sb_{kp}_{ci}")
                     for ci, (off, cw) in enumerate(chunks)]
                    for kp in range(KP)
                ]
                for kp in range(KP):
                    for ci, (off, cw) in enumerate(chunks):
                        nc.sync.dma_start(
                            sb[kp][ci][:],
                            src_t[:, 2 * kp:2 * kp + 2, off:off + cw])
                return sb

            x_hi = load_x(xh_t, "xh")
            x_lo = load_x(xl_t, "xl")

            y_sb = ypool.tile([P, HO, C], F32)
            act_h = actpool.tile([P, FG, C], F8, tag="act_h")
            act_l = actpool.tile([P, FG, C], F8, tag="act_l")

            w2_tiles = {}

            def load_w2(g, ho):
                t_h = w2pool.tile([P, FG, P], F8, tag="w2h")
                nc.sync.dma_start(
                    t_h[:], w2h_t[:, g * FG:(g + 1) * FG, ho * P:(ho + 1) * P])
                t_l = w2pool.tile([P, FG, P], F8, tag="w2l")
                nc.sync.dma_start(
                    t_l[:], w2l_t[:, g * FG:(g + 1) * FG, ho * P:(ho + 1) * P])
                w2_tiles[(g, ho)] = (t_h, t_l)

            for g in range(n_groups):
                f0 = g * FG
                # ---- up + gate projections and SwiGLU for this f-group ----
                for fi in range(FG):
                    fo = f0 + fi
                    if fo + 2 < FO:
                        load_w13(fo + 2)
                    w1h_f, w1l_f, w3h_f, w3l_f = w13_tiles.pop(fo)
                    for ci, (off, cw) in enumerate(chunks):
                        h_ps = ps_h.tile([P, 512], F32)
                        u_ps = ps_u.tile([P, 512], F32)
                        # 3-term compensated fp8 product, 2 k-tiles/instr
                        for psum, wh, wl in ((h_ps, w1h_f, w1l_f),
                                             (u_ps, w3h_f, w3l_f)):
                            idx = 0
                            for wt, xt_ in ((wh, x_hi), (wl, x_hi),
                                            (wh, x_lo)):
                                for kp in range(KP):
                                    nc.tensor.matmul(
                                        psum[:, :cw],
                                        wt[:, 2 * kp:2 * kp + 2],
                                        xt_[kp][ci][:],
                                        start=(idx == 0),
                                        stop=(idx == 3 * KP - 1),
                                        perf_mode=DR,
                                    )
                                    idx += 1
                        # silu removes the w1 scale; u keeps the w3 scale so
                        # a32 = 16 * silu(h) * u, quantized to fp8 hi + lo
                        s_sb = tmppool.tile([P, 512], F32, tag="silu")
                        nc.scalar.activation(
                            s_sb[:, :cw], h_ps[:, :cw],
                            mybir.ActivationFunctionType.Silu,
                            scale=1.0 / SW1,
                        )
                        a32 = tmppool.tile([P, 512], F32, tag="a32")
                        nc.vector.tensor_mul(
                            a32[:, :cw], s_sb[:, :cw], u_ps[:, :cw])
                        nc.scalar.activation(
                            act_h[:, fi, off:off + cw], a32[:, :cw],
                            mybir.ActivationFunctionType.Copy,
                        )
                        nc.vector.tensor_sub(
                            act_l[:, fi, off:off + cw],
                            a32[:, :cw], act_h[:, fi, off:off + cw])
                # ---- down projection: y += act_g @ w2[f-group] ----
                load_w2(g, 0)
                for ho in range(HO):
                    if ho + 1 < HO:
                        load_w2(g, ho + 1)
                    elif g + 1 < n_groups:
                        load_w2(g + 1, 0)
                    w2h_h, w2l_h = w2_tiles.pop((g, ho))
                    for ci, (off, cw) in enumerate(chunks):
                        y_ps = ps_y.tile([P, 512], F32)
                        idx = 0
                        for wt, at in ((w2h_h, act_h), (w2l_h, act_h),
                                       (w2h_h, act_l)):
                            for j in range(FP):
                                nc.tensor.matmul(
                                    y_ps[:, :cw],
                                    wt[:, 2 * j:2 * j + 2],
                                    at[:, 2 * j:2 * j + 2, off:off + cw],
                                    start=(idx == 0),
                                    stop=(idx == 3 * FP - 1),
                                    perf_mode=DR,
                                )
                                idx += 1
                        if g == 0:
                            nc.vector.tensor_copy(
                                y_sb[:, ho, off:off + cw], y_ps[:, :cw])
                        else:
                            nc.vector.tensor_add(
                                y_sb[:, ho, off:off + cw],
                                y_sb[:, ho, off:off + cw], y_ps[:, :cw])
                        if g == n_groups - 1:
                            # final contribution: store while the remaining
                            # tiles are still accumulating
                            nc.sync.dma_start(yT_t[:, ho, off:off + cw],
                                              y_sb[:, ho, off:off + cw])

    nc.compile()
    return nc


def _route(x, gate_w):
    """Host-side gate: returns token index list and combine weight per expert."""
    xt = x.reshape(-1, H)
    scores = xt.astype(np.float64) @ gate_w.astype(np.float64).T
    ei = np.argsort(-scores, axis=1, kind="stable")[:, :TOPK]  # [T, 2]
    ev = np.take_along_axis(scores, ei, axis=1)                # [T, 2]
    ev = ev - ev.max(axis=1, keepdims=True)
    ew = np.exp(ev)
    ew = ew / ew.sum(axis=1, keepdims=True)                    # softmax [T, 2]
    routes = []
    for e in range(E):
        mask = ei == e                                         # [T, 2]
        toks = np.nonzero(mask.any(axis=1))[0]
        wts = (ew * mask).sum(axis=1)[toks]
        routes.append((toks, wts.astype(np.float32)))
    return routes


def _split8(v):
    """hi = fp8(v), lo = fp8(v - hi); both as fp8 arrays."""
    hi = v.astype(FP8)
    lo = (v - hi.astype(np.float32)).astype(FP8)
    return hi, lo


def _run(inputs, trace=False, trace_kwargs=None):
    x = np.ascontiguousarray(np.asarray(inputs["x"], dtype=np.float32))
    gate_w = np.asarray(inputs["gate_w"], dtype=np.float32)
    w1 = np.asarray(inputs["w1"], dtype=np.float32)
    w3 = np.asarray(inputs["w3"], dtype=np.float32)
    w2 = np.asarray(inputs["w2"], dtype=np.float32)
    B, S, Hd = x.shape
    assert Hd == H and w1.shape == (E, H, F) and w2.shape == (E, F, H)

    routes = _route(x, gate_w)
    max_count = max(len(toks) for toks, _ in routes)
    C = max(256, math.ceil(max_count / 16) * 16)

    if C not in _NC_CACHE:
        _NC_CACHE[C] = _build_nc(C)
    nc = _NC_CACHE[C]

    xt = x.reshape(-1, H)
    in_maps = []
    for e in range(E):
        toks, _ = routes[e]
        xT_e = np.zeros((H, C), dtype=np.float32)
        xT_e[:, :len(toks)] = xt[toks].T
        xh_e, xl_e = _split8(xT_e)
        w1h_e, w1l_e = _split8(SW1 * w1[e])
        w3h_e, w3l_e = _split8(SW3 * w3[e])
        w2h_e, w2l_e = _split8(SW2 * w2[e])
        in_maps.append({
            "xh": xh_e, "xl": xl_e,
            "w1h": w1h_e, "w1l": w1l_e,
            "w3h": w3h_e, "w3l": w3l_e,
            "w2h": w2h_e, "w2l": w2l_e,
        })

    res = run_bass_kernel_spmd(
        nc, in_maps, core_ids=list(range(N_CORES)),
        trace=trace, trace_kwargs=trace_kwargs or {},
    )

    y = np.zeros((B * S, H), dtype=np.float32)
    for e in range(E):
        toks, wts = routes[e]
        yT_e = res.results[e]["yT"]  # [H, C]
        y[toks] += (Y_DESCALE * wts)[:, None] * yT_e[:, :len(toks)].T
    return y.reshape(B, S, H), res


def kernel(**inputs):
    y, _ = _run(inputs)
    return y


# revision 26
# speedup vs baseline: 1.3500x; 1.0498x over previous
"""MoE (top-2 of 8 experts, SwiGLU MLP) on 8 Trainium2 NeuronCores.

Strategy (expert-parallel, host-side routing, fp8 DoubleRow matmuls):
  - Host computes the gate (scores -> top-2 -> softmax) in f64; the rank-2/3
    score gap is >1e-4 for these inputs so selection is rounding-robust.
  - Core e receives the tokens routed to expert e (transposed to [H, C],
    zero-padded to capacity C) plus expert e's w1/w3/w2.
  - All matmuls run as fp8e4m3 DoubleRow (two 128-deep k-tiles per
    instruction at 0.5 PE cycles per output row = 4x the fp32r MAC rate).
    Plain fp8 quantization is far too coarse (~6e-2 rel err), so every
    operand is split into hi + lo fp8 parts (lo = fp8 of the quantization
    residual) and each product uses 3 of the 4 cross terms:
        a @ w  ~=  a_hi @ w_hi + a_hi @ w_lo + a_lo @ w_hi
    which lands at ~2e-3 rel err (measured) at 0.75 cycles per 128-deep
    output row -- a ~1.33x PE speedup over the fp32r kernel.
  - Weights are pre-scaled by powers of 2 (w1*64, w3*16, w2*64) so their
    0.02-sigma values sit in e4m3's normal range instead of the subnormal
    range (which is what ruins plain fp8 here).  The w1 scale is removed
    by the Silu activation's input scale; the w3 and w2 scales ride
    through the (linear) down projection and are divided out on the host.
  - Host pre-packs each weight stream into per-f-tile contiguous blocks
    (4KB per partition per DMA) so the DMA cost model's small-run penalty
    (2x under 512B) and per-descriptor floors never bite.
  - Each core streams expert weights from HBM once (25MB fp8 vs 50MB f32),
    keeps x, act, y resident in SBUF; act is quantized on-chip to fp8
    hi + lo at scale 16 (absmax(16*act) ~ 114 < 240 = e4m3 max).
  - Host scatter-adds the weighted per-expert outputs back to [B, S, H].

Hardcoded problem shapes: x [2, 2048, 1024], E=8 experts, top-2,
w1/w3 [8, 1024, 4096], w2 [8, 4096, 1024].
"""

import math

import ml_dtypes
import numpy as np

import concourse.bass as bass  # noqa: F401  (registers AP machinery)
import concourse.tile as tile
from concourse import bacc, mybir
from concourse.bass_utils import run_bass_kernel_spmd

P = 128
H = 1024
F = 4096
E = 8
TOPK = 2
N_CORES = 8

KO = H // P  # 8 contraction tiles for the up/gate projections
FO = F // P  # 32 intermediate tiles
HO = H // P  # 8 output tiles
FG = 16      # f-tiles per down-projection group (FO/FG = 2 groups)
NG = FO // FG

SW1 = 64.0   # w1 pre-scale (removed by the Silu input scale)
SW3 = 16.0   # w3 pre-scale => act is produced at scale 16
SW2 = 64.0   # w2 pre-scale
Y_DESCALE = 1.0 / (SW3 * SW2)  # folded into the host combine weights

F32 = mybir.dt.float32
F8 = mybir.dt.float8e4
FP8 = ml_dtypes.float8_e4m3
DR = mybir.MatmulPerfMode.DoubleRow

_NC_CACHE: dict = {}


def _chunks(C: int):
    """Full 512-wide chunks plus one remainder: 512B+ contiguous DMA runs
    and one PSUM bank per chunk."""
    assert C % 16 == 0
    out, off = [], 0
    while off < C:
        cw = min(512, C - off)
        out.append((off, cw))
        off += cw
    return out


def _build_nc(C: int):
    chunks = _chunks(C)
    KP = KO // 2   # DoubleRow k-tile pairs in the up projections
    FP_ = FG // 2  # DoubleRow f-tile pairs per down-projection group

    nc = bacc.Bacc("TRN2", target_bir_lowering=False, debug=False,
                   num_devices=N_CORES)
    # x hi/lo: [P, KO*C], row p holds h = ko*P + p
    xh = nc.dram_tensor("xh", [P, KO * C], F8, kind="ExternalInput").ap()
    xl = nc.dram_tensor("xl", [P, KO * C], F8, kind="ExternalInput").ap()
    # packed weights: one contiguous [P, 4KB] block per f-tile
    w13p = nc.dram_tensor("w13p", [FO * P, 4 * KO * P], F8,
                          kind="ExternalInput").ap()
    w2p = nc.dram_tensor("w2p", [NG * HO * P, 2 * FG * P], F8,
                         kind="ExternalInput").ap()
    yT = nc.dram_tensor("yT", [H, C], F32, kind="ExternalOutput").ap()

    xh_t = xh.rearrange("p (ko c) -> p ko c", ko=KO)
    xl_t = xl.rearrange("p (ko c) -> p ko c", ko=KO)
    yT_t = yT.rearrange("(ho p) c -> p ho c", p=P)        # [128, HO, C]

    with tile.TileContext(nc) as tc:
        with (
            tc.tile_pool(name="xres", bufs=1) as xpool,
            tc.tile_pool(name="yres", bufs=1) as ypool,
            tc.tile_pool(name="actres", bufs=1) as actpool,
            tc.tile_pool(name="w13", bufs=4) as w13pool,
            tc.tile_pool(name="w2p", bufs=2) as w2pool,
            tc.tile_pool(name="tmp", bufs=3) as tmppool,
            tc.tile_pool(name="psh", bufs=3, space="PSUM") as ps_h,
            tc.tile_pool(name="psu", bufs=3, space="PSUM") as ps_u,
            tc.tile_pool(name="psy", bufs=2, space="PSUM") as ps_y,
        ):
            w13_tiles = {}

            def load_w13(fo, eng=None):
                # one DMA per f-tile: [P, (w1h|w1l|w3h|w3l), KO, 128]
                t = w13pool.tile([P, 4, KO, P], F8, tag="w13",
                                 name=f"w13_f{fo}")
                (eng or nc.sync).dma_start(t[:], w13p[fo * P:(fo + 1) * P, :])
                w13_tiles[fo] = t

            # DMA-belt order tuned for the earliest possible PE start:
            # f-tile 0's w1h part alone (one 1KB-per-partition DMA on SP),
            # then x chunk 0 / x rest (hi before lo -- the x-corr matmuls
            # come last in each group), then the remaining f-tiles, all on
            # the Activation queue so the SP loop prefetches (gated by the
            # w13 ring) can't race them onto the belt
            w13_f0 = w13pool.tile([P, 4, KO, P], F8, tag="w13",
                                  name="w13_f0")
            KB = KO * P
            w13_tiles[0] = w13_f0
            c0 = chunks[0][1]
            x_hi_c0 = xpool.tile([P, KO, c0], F8, tag="xhc0", name="xh_c0")
            x_lo_c0 = xpool.tile([P, KO, c0], F8, tag="xlc0", name="xl_c0")
            x_hi_r = x_lo_r = None
            if C > c0:
                x_hi_r = xpool.tile([P, KO, C - c0], F8, tag="xhr",
                                    name="xh_r")
                x_lo_r = xpool.tile([P, KO, C - c0], F8, tag="xlr",
                                    name="xl_r")
            # single queue, exact consumption order of the first f-tile's
            # reordered (main -> w-corr -> x-corr) accumulation groups
            nc.scalar.dma_start(w13_f0[:, 0], w13p[0:P, 0:KB])
            nc.scalar.dma_start(x_hi_c0[:], xh_t[:, :, 0:c0])
            nc.scalar.dma_start(w13_f0[:, 2], w13p[0:P, 2 * KB:3 * KB])
            if C > c0:
                nc.scalar.dma_start(x_hi_r[:], xh_t[:, :, c0:C])
            nc.scalar.dma_start(w13_f0[:, 1], w13p[0:P, KB:2 * KB])
            nc.scalar.dma_start(w13_f0[:, 3], w13p[0:P, 3 * KB:4 * KB])
            nc.scalar.dma_start(x_lo_c0[:], xl_t[:, :, 0:c0])
            if C > c0:
                nc.scalar.dma_start(x_lo_r[:], xl_t[:, :, c0:C])
            for fo in range(1, 4):
                load_w13(fo, eng=nc.scalar)

            def x_slice(hi, ci, kp):
                """moving operand [P, 2, cw] for chunk ci, k-pair kp"""
                off, cw = chunks[ci]
                if ci == 0:
                    t = x_hi_c0 if hi else x_lo_c0
                    return t[:, 2 * kp:2 * kp + 2, :]
                t = x_hi_r if hi else x_lo_r
                return t[:, 2 * kp:2 * kp + 2, off - c0:off - c0 + cw]


            y_sb = ypool.tile([P, HO, C], F32)
            act_h = actpool.tile([P, FG, C], F8, tag="act_h")
            act_l = actpool.tile([P, FG, C], F8, tag="act_l")

            w2_tiles = {}

            def load_w2(g, ho):
                # Activation DMA queue: keeps w2 prefetches out of the SP
                # queue, whose sequencer blocks on the y-store sem waits
                t = w2pool.tile([P, 2, FG, P], F8, tag="w2",
                                name=f"w2_g{g}h{ho}")
                r0 = (g * HO + ho) * P
                nc.scalar.dma_start(t[:], w2p[r0:r0 + P, :])
                w2_tiles[(g, ho)] = t

            for g in range(NG):
                f0 = g * FG
                load_w2(g, 0)
                # ---- up + gate projections and SwiGLU for this f-group ----
                for fi in range(FG):
                    fo = f0 + fi
                    if fo + 4 < FO:
                        load_w13(fo + 4)
                    w13_f = w13_tiles.pop(fo)

                    def up_mm(psum, part, hi, ci, start, stop):
                        cw = chunks[ci][1]
                        for kp in range(KP):
                            nc.tensor.matmul(
                                psum[:, :cw],
                                w13_f[:, part, 2 * kp:2 * kp + 2],
                                x_slice(hi, ci, kp),
                                start=start and kp == 0,
                                stop=stop and kp == KP - 1,
                                perf_mode=DR,
                            )

                    def consume(h_ps, u_ps, ci):
                        # silu removes the w1 scale; u keeps the w3 scale so
                        # a32 = 16 * silu(h) * u, quantized to fp8 hi + lo
                        off, cw = chunks[ci]
                        s_sb = tmppool.tile([P, 512], F32, tag="silu")
                        nc.scalar.activation(
                            s_sb[:, :cw], h_ps[:, :cw],
                            mybir.ActivationFunctionType.Silu,
                            scale=1.0 / SW1,
                        )
                        a32 = tmppool.tile([P, 512], F32, tag="a32")
                        nc.vector.tensor_mul(
                            a32[:, :cw], s_sb[:, :cw], u_ps[:, :cw])
                        nc.scalar.activation(
                            act_h[:, fi, off:off + cw], a32[:, :cw],
                            mybir.ActivationFunctionType.Copy,
                        )
                        nc.vector.tensor_sub(
                            act_l[:, fi, off:off + cw],
                            a32[:, :cw], act_h[:, fi, off:off + cw])

                    if fo == 0:
                        # first f-tile: emit the main terms for every chunk
                        # first, then the w corrections, then the x_lo
                        # corrections -- the PE starts as soon as w1h and
                        # x_hi[chunk 0] land, while the remaining operands
                        # are still in flight on the DMA belt
                        psums = []
                        for ci in range(len(chunks)):
                            h_ps = ps_h.tile([P, 512], F32, tag="ps",
                                             name="h_ps")
                            u_ps = ps_u.tile([P, 512], F32)
                            psums.append((h_ps, u_ps))
                            up_mm(h_ps, 0, True, ci, start=True, stop=False)
                            up_mm(u_ps, 2, True, ci, start=True, stop=False)
                        for ci in range(len(chunks)):
                            h_ps, u_ps = psums[ci]
                            up_mm(h_ps, 1, True, ci, start=False, stop=False)
                            up_mm(u_ps, 3, True, ci, start=False, stop=False)
                        for ci in range(len(chunks)):
                            h_ps, u_ps = psums[ci]
                            up_mm(h_ps, 0, False, ci, start=False, stop=True)
                            up_mm(u_ps, 2, False, ci, start=False, stop=True)
                            consume(h_ps, u_ps, ci)
                        continue

                    for ci, (off, cw) in enumerate(chunks):
                        h_ps = ps_h.tile([P, 512], F32, tag="ps", name="h_ps")
                        u_ps = ps_u.tile([P, 512], F32)
                        # 3-term compensated fp8 product, 2 k-tiles/instr
                        for psum, hi_part, lo_part in ((h_ps, 0, 1),
                                                       (u_ps, 2, 3)):
                            up_mm(psum, hi_part, True, ci,
                                  start=True, stop=False)
                            up_mm(psum, lo_part, True, ci,
                                  start=False, stop=False)
                            up_mm(psum, hi_part, False, ci,
                                  start=False, stop=True)
                        consume(h_ps, u_ps, ci)
                # ---- down projection: y += act_g @ w2[f-group] ----
                for ho in range(HO):
                    if ho + 1 < HO:
                        load_w2(g, ho + 1)
                    w2_gh = w2_tiles.pop((g, ho))
                    for ci, (off, cw) in enumerate(chunks):
                        # alternate with the (idle during down) h pool for an
                        # effectively deeper y PSUM ring
                        if (ho * len(chunks) + ci) % 2:
                            y_ps = ps_y.tile([P, 512], F32)
                        else:
                            y_ps = ps_h.tile([P, 512], F32, tag="ps",
                                             name="h_ps")
                        idx = 0
                        for part, at in ((0, act_h), (1, act_h), (0, act_l)):
                            for j in range(FP_):
                                nc.tensor.matmul(
                                    y_ps[:, :cw],
                                    w2_gh[:, part, 2 * j:2 * j + 2],
                                    at[:, 2 * j:2 * j + 2, off:off + cw],
                                    start=(idx == 0),
                                    stop=(idx == 3 * FP_ - 1),
                                    perf_mode=DR,
                                )
                                idx += 1
                        if g == 0:
                            nc.vector.tensor_copy(
                                y_sb[:, ho, off:off + cw], y_ps[:, :cw])
                        else:
                            nc.vector.tensor_add(
                                y_sb[:, ho, off:off + cw],
                                y_sb[:, ho, off:off + cw], y_ps[:, :cw])
                        if g == NG - 1:
                            # final contribution: store while the remaining
                            # tiles are still accumulating
                            nc.sync.dma_start(yT_t[:, ho, off:off + cw],
                                              y_sb[:, ho, off:off + cw])

    nc.compile()
    return nc


def _route(x, gate_w):
    """Host-side gate: returns token index list and combine weight per expert."""
    xt = x.reshape(-1, H)
    scores = xt.astype(np.float64) @ gate_w.astype(np.float64).T
    ei = np.argsort(-scores, axis=1, kind="stable")[:, :TOPK]  # [T, 2]
    ev = np.take_along_axis(scores, ei, axis=1)                # [T, 2]
    ev = ev - ev.max(axis=1, keepdims=True)
    ew = np.exp(ev)
    ew = ew / ew.sum(axis=1, keepdims=True)                    # softmax [T, 2]
    routes = []
    for e in range(E):
        mask = ei == e                                         # [T, 2]
        toks = np.nonzero(mask.any(axis=1))[0]
        wts = (ew * mask).sum(axis=1)[toks]
        routes.append((toks, wts.astype(np.float32)))
    return routes


def _split8(v):
    """hi = fp8(v), lo = fp8(v - hi); both as fp8 arrays."""
    hi = v.astype(FP8)
    lo = (v - hi.astype(np.float32)).astype(FP8)
    return hi, lo


def _pack_w13(w1_e, w3_e):
    """[FO*P, 4*KO*P]: per f-tile fo, per partition p, the 4KB block
    [part(w1h|w1l|w3h|w3l), ko, j] with element Wpart[ko*P + p, fo*P + j]."""
    w1h, w1l = _split8(SW1 * w1_e)
    w3h, w3l = _split8(SW3 * w3_e)
    w4 = np.stack([w1h, w1l, w3h, w3l])               # [4, H, F]
    w4 = w4.reshape(4, KO, P, FO, P)                  # [part, ko, p, fo, j]
    return np.ascontiguousarray(
        w4.transpose(3, 2, 0, 1, 4).reshape(FO * P, 4 * KO * P))


def _pack_w2(w2_e):
    """[NG*HO*P, 2*FG*P]: per (g, ho), per partition p, the 4KB block
    [part(hi|lo), fj, j] with element W2part[(g*FG+fj)*P + p, ho*P + j]."""
    w2h, w2l = _split8(SW2 * w2_e)
    w2s = np.stack([w2h, w2l])                        # [2, F, H]
    w2s = w2s.reshape(2, NG, FG, P, HO, P)            # [part, g, fj, p, ho, j]
    return np.ascontiguousarray(
        w2s.transpose(1, 4, 3, 0, 2, 5).reshape(NG * HO * P, 2 * FG * P))


def _pack_x(xT_e):
    """[P, KO*C] fp8 hi/lo: row p holds h = ko*P + p (runs of C per ko)."""
    xh, xl = _split8(xT_e)                            # [H, C]
    C = xT_e.shape[1]
    xh = xh.reshape(KO, P, C).transpose(1, 0, 2).reshape(P, KO * C)
    xl = xl.reshape(KO, P, C).transpose(1, 0, 2).reshape(P, KO * C)
    return np.ascontiguousarray(xh), np.ascontiguousarray(xl)


def _run(inputs, trace=False, trace_kwargs=None):
    x = np.ascontiguousarray(np.asarray(inputs["x"], dtype=np.float32))
    gate_w = np.asarray(inputs["gate_w"], dtype=np.float32)
    w1 = np.asarray(inputs["w1"], dtype=np.float32)
    w3 = np.asarray(inputs["w3"], dtype=np.float32)
    w2 = np.asarray(inputs["w2"], dtype=np.float32)
    B, S, Hd = x.shape
    assert Hd == H and w1.shape == (E, H, F) and w2.shape == (E, F, H)

    routes = _route(x, gate_w)
    max_count = max(len(toks) for toks, _ in routes)
    C = max(256, math.ceil(max_count / 16) * 16)

    if C not in _NC_CACHE:
        _NC_CACHE[C] = _build_nc(C)
    nc = _NC_CACHE[C]

    xt = x.reshape(-1, H)
    in_maps = []
    for e in range(E):
        toks, _ = routes[e]
        xT_e = np.zeros((H, C), dtype=np.float32)
        xT_e[:, :len(toks)] = xt[toks].T
        xh_e, xl_e = _pack_x(xT_e)
        in_maps.append({
            "xh": xh_e, "xl": xl_e,
            "w13p": _pack_w13(w1[e], w3[e]),
            "w2p": _pack_w2(w2[e]),
        })

    res = run_bass_kernel_spmd(
        nc, in_maps, core_ids=list(range(N_CORES)),
        trace=trace, trace_kwargs=trace_kwargs or {},
    )

    y = np.zeros((B * S, H), dtype=np.float32)
    for e in range(E):
        toks, wts = routes[e]
        yT_e = res.results[e]["yT"]  # [H, C]
        y[toks] += (Y_DESCALE * wts)[:, None] * yT_e[:, :len(toks)].T
    return y.reshape(B, S, H), res


def kernel(**inputs):
    y, _ = _run(inputs)
    return y
